# revision 25
# baseline (speedup 1.0000x reference)
"""Trainium2 Bass kernel for nn_CustomGNN_66881230733874 (2-layer GAT + mean-pool + MLP).

Sharding: data-parallel over batch B=8 -> one graph per NeuronCore (8 cores).
Each core computes its full graph end-to-end (no collectives); host gathers [8,1].

Layout strategy (per core):
  - Activations live feature-on-partition ("transposed"): XT [F, N], QT/KT [(h,d), N].
  - Scores computed directly transposed: S^T[m, n] = K_h Q_h^T via PE.
  - softmax without max-subtraction (scores are O(1) by construction);
    masked entries are exactly zeroed by multiplying exp(S) with (1-A)^T.
  - ctx_unnorm [n, u] via lhsT=E[m,n-chunk], rhs=V_aug[m, u+1] where V_aug has a
    ones column -> last PSUM column accumulates the softmax denominator Z[n].
  - normalize with per-partition reciprocal (DVE), pack bf16, DMA-transpose
    (SBUF xbar) to get ctx^T [(h,u), N] for the W_out matmul. No PE transposes.
  - Weight matmuls use split-bf16 (hi+lo) for the value path (Wv, Wo) to kill
    correlated rounding error; Wq/Wk single-bf16 (score path is tolerant).
"""

import numpy as np

import concourse.bass as bass
import concourse.mybir as mybir
import concourse.tile as tile
from concourse import bacc
from concourse.bass_utils import run_bass_kernel_spmd
from concourse.masks import make_identity

F32 = mybir.dt.float32
BF16 = mybir.dt.bfloat16
AF = mybir.ActivationFunctionType
OP = mybir.AluOpType

B = 8
N = 1024
F = 64
H = 8
U1, U2 = 128, 64
NT = N // 128  # 8 node chunks

WEIGHT_NAMES = [
    "Wq1", "Wk1", "Wv1", "Wo1", "Wq2", "Wk2", "Wv2", "Wo2",
    "W1", "b1", "W2", "b2", "W3", "b3",
]


def _load_weight2d(nc, sb, scratch, name, dram, part, cols, split):
    """DMA a [part, cols] f32 weight to SBUF bf16 (hi[, lo])."""
    w32 = scratch.tile([part, cols], F32, tag="wscratch", bufs=2)
    nc.sync.dma_start(out=w32, in_=dram[:])
    hi = sb.tile([part, cols], BF16, tag=f"w_{name}_hi", bufs=1)
    nc.vector.tensor_copy(out=hi, in_=w32)
    lo = None
    if split:
        lo = sb.tile([part, cols], BF16, tag=f"w_{name}_lo", bufs=1)
        nc.vector.tensor_sub(out=lo, in0=w32, in1=hi)
    return hi, lo


def _load_weight_kt(nc, sb, scratch, name, dram, kt, cols, split):
    """DMA a [kt*128, cols] f32 weight to SBUF bf16 [128, kt, cols] (hi, lo)."""
    w32 = scratch.tile([128, kt, cols], F32, tag="wscratch", bufs=2)
    nc.sync.dma_start(out=w32, in_=dram[:].rearrange("(k p) c -> p k c", p=128))
    hi = sb.tile([128, kt, cols], BF16, tag=f"w_{name}_hi", bufs=1)
    nc.vector.tensor_copy(out=hi, in_=w32)
    lo = sb.tile([128, kt, cols], BF16, tag=f"w_{name}_lo", bufs=1)
    nc.vector.tensor_sub(out=lo, in0=w32, in1=hi)
    return hi, lo


def _gat_layer(nc, sb, ps_big, ps_small, XT, U, Uout,
               Wq, Wk, Wv_hi, Wv_lo, Wo_hi, Wo_lo, Mt, HT_out,
               deferred_cb=None):
    """One GAT layer. XT: bf16 [Cin, N]. Writes HT_out: bf16 [Uout, N]."""
    HU = H * U
    DT = HU // 128          # number of 128-row tiles of QT/KT
    UD = 128 // U           # heads per QT/KT tile
    VW = U + 1              # V_aug width per head
    inv_sqrt_u = 1.0 / float(np.sqrt(U))

    # ---- projections: QT/KT [(h,d), N] bf16; V_aug [n, H*(U+1)] bf16 ----
    QT = sb.tile([128, DT, N], BF16, tag="QT", bufs=1)
    KT = sb.tile([128, DT, N], BF16, tag="KT", bufs=1)
    VA = sb.tile([128, NT, H * VW], BF16, tag="VA", bufs=1)

    def qk_proj(d, tag="ps_p", pool=None):
        for w, dst, on_act in ((Wq, QT, True), (Wk, KT, False)):
            for q in range(2):
                p = (pool or ps_big).tile([128, 512], F32, tag=tag, bufs=2)
                nc.tensor.matmul(p, lhsT=w[:, d * 128:(d + 1) * 128],
                                 rhs=XT[:, q * 512:(q + 1) * 512],
                                 start=True, stop=True)
                if on_act:
                    nc.scalar.copy(out=dst[:, d, q * 512:(q + 1) * 512], in_=p)
                else:
                    nc.vector.tensor_copy(out=dst[:, d, q * 512:(q + 1) * 512],
                                          in_=p)

    def v_proj():
        n_vc = HU // 512  # 512-wide chunks of H*U
        hpc = 512 // U    # heads per 512 chunk
        for m in range(NT):
            VAm = VA[:, m, :].rearrange("p (h x) -> p h x", h=H)
            nc.vector.memset(VAm[:, :, U:U + 1], 1.0)
        for c in range(n_vc):
            for m in range(NT):
                VAm = VA[:, m, :].rearrange("p (h x) -> p h x", h=H)
                p = ps_big.tile([128, 512], F32, tag="ps_p", bufs=2)
                xm = XT[:, m * 128:(m + 1) * 128]
                nc.tensor.matmul(p, lhsT=xm, rhs=Wv_hi[:, c * 512:(c + 1) * 512],
                                 start=True, stop=False)
                nc.tensor.matmul(p, lhsT=xm, rhs=Wv_lo[:, c * 512:(c + 1) * 512],
                                 start=False, stop=True)
                nc.vector.tensor_copy(
                    out=VAm[:, c * hpc:(c + 1) * hpc, 0:U],
                    in_=p.rearrange("p (h x) -> p h x", h=hpc),
                )

    # ---- attention, software-pipelined: S(h+1) emitted before ctx(h) so the
    # exp stream on ACT is never starved by PE's ctx block. m-chunks are
    # processed in pairs (FD=2048) to halve per-instruction overheads ----
    CT = sb.tile([128, DT, N], BF16, tag="CT", bufs=1)  # ctx^T [(h,u), N]
    E_tiles = {}
    cp2_tiles = {}
    wo_psum = [None, None]
    KD = HU // 128

    def s_phase(h):
        kt_i, k_off = h // UD, (h % UD) * U
        E = sb.tile([128, NT, N], BF16, tag="E", bufs=4)
        E_tiles[h] = E
        for m in range(NT):
            s = ps_big.tile([128, 1024], F32, tag="ps_s", bufs=2)
            for q in range(2):
                nc.tensor.matmul(
                    s[:, q * 512:(q + 1) * 512],
                    lhsT=KT[k_off:k_off + U, kt_i, m * 128:(m + 1) * 128],
                    rhs=QT[k_off:k_off + U, kt_i, q * 512:(q + 1) * 512],
                    start=True, stop=True,
                )
            nc.scalar.activation(out=E[:, m, :], in_=s, func=AF.Exp,
                                 scale=inv_sqrt_u)
            nc.vector.tensor_mul(out=E[:, m, :], in0=E[:, m, :], in1=Mt[:, m, :])

    GH = 2                    # heads per transpose/Wo group
    TPG = GH * U // 128       # CT tiles per group

    def ctx_phase(h):
        E = E_tiles.pop(h)
        g, hg = divmod(h, GH)
        batched_t = (U == 128)  # head-major cpq -> one xbar transpose per group
        if hg == 0:
            if batched_t:
                cpq_new = sb.tile([128, GH, NT, U], BF16, tag="cpq", bufs=2)
            else:
                cpq_new = sb.tile([128, NT, GH * U], BF16, tag="cpq", bufs=2)
            cp2_tiles[g] = cpq_new
        cpq = cp2_tiles[g]
        for nn in range(NT):
            c = ps_small.tile([128, VW], F32, tag="ps_c", bufs=2)
            for m in range(NT):
                nc.tensor.matmul(
                    c,
                    lhsT=E[:, m, nn * 128:(nn + 1) * 128],
                    rhs=VA[:, m, h * VW:(h + 1) * VW],
                    start=(m == 0), stop=(m == NT - 1),
                )
            r = sb.tile([128, 1], F32, tag="recip", bufs=8)
            nc.vector.reciprocal(out=r, in_=c[:, U:U + 1])
            dst = (cpq[:, hg, nn, :] if batched_t
                   else cpq[:, nn, hg * U:(hg + 1) * U])
            nc.vector.tensor_scalar(out=dst, in0=c[:, 0:U], scalar1=r,
                                    scalar2=None, op0=OP.mult)
        # one xbar transpose per group: free index c lands at
        # out[c%128, c//128, p]; both layouts put (head-row, tile) in
        # (c%128 resp. c//128) so the same CT view works.
        if hg == GH - 1:
            nc.sync.dma_start(
                out=CT[:, g * TPG:(g + 1) * TPG, :].rearrange(
                    "p k (t c) -> p k t c", c=128),
                in_=(cpq[:, :, :, :] if batched_t else cpq[:, :, :]),
                transpose=True)
        # fold the Wo accumulation once this group's CT tiles are complete
        if hg == GH - 1:
            first, last = (g == 0), (g == H // GH - 1)
            for q in range(2):
                if first:
                    wp = ps_big.tile([Uout, 512], F32, tag="ps_p", bufs=2)
                    wo_psum[q] = wp
                for ki in range(TPG):
                    k = g * TPG + ki
                    rhs = CT[:, k, q * 512:(q + 1) * 512]
                    nc.tensor.matmul(wo_psum[q], lhsT=Wo_hi[:, k, :], rhs=rhs,
                                     start=(first and ki == 0), stop=False)
                    nc.tensor.matmul(wo_psum[q], lhsT=Wo_lo[:, k, :], rhs=rhs,
                                     start=False, stop=(last and ki == TPG - 1))
            if last:
                for q in range(2):
                    nc.scalar.copy(out=HT_out[:, q * 512:(q + 1) * 512],
                                   in_=wo_psum[q])

    # head-0's QK projection chunk first so its S/exp stream starts early;
    # the remaining projections overlap the first heads' attention
    emitted_qk = set()

    def qk_if_needed(h, **kw):
        d = h // UD
        if h < H and d not in emitted_qk:
            emitted_qk.add(d)
            qk_proj(d, **kw)

    qk_if_needed(0)
    qk_if_needed(1)
    v_proj()
    qk_if_needed(2, tag="ps_c", pool=ps_small)
    s_phase(0)
    s_phase(1)
    for h in range(H):
        if deferred_cb is not None:
            deferred_cb(h)
        if h + 2 < H:
            qk_if_needed(h + 3, tag="ps_c", pool=ps_small)
            s_phase(h + 2)
        ctx_phase(h)


def build_nc(repeats=1):
    nc = bacc.Bacc("TRN2", target_bir_lowering=False, debug=False)

    x_d = nc.dram_tensor("X", [N, F], F32, kind="ExternalInput")
    a_d = nc.dram_tensor("A", [N, N], F32, kind="ExternalInput")
    w_d = {}
    shapes = {
        "Wq1": [F, H * U1], "Wk1": [F, H * U1], "Wv1": [F, H * U1],
        "Wo1": [H * U1, U1],
        "Wq2": [U1, H * U2], "Wk2": [U1, H * U2], "Wv2": [U1, H * U2],
        "Wo2": [H * U2, U2],
        "W1": [F, 32], "b1": [32], "W2": [32, 16], "b2": [16],
        "W3": [16, 1], "b3": [1],
    }
    for k, s in shapes.items():
        w_d[k] = nc.dram_tensor(k, s, F32, kind="ExternalInput")
    y_d = nc.dram_tensor("y", [1, 1], F32, kind="ExternalOutput")

    with tile.TileContext(nc) as tc:
        with (
            tc.tile_pool(name="sb", bufs=1) as sb,
            tc.tile_pool(name="scratch", bufs=2) as scratch,
            tc.tile_pool(name="ps_big", bufs=2, space="PSUM") as ps_big,
            tc.tile_pool(name="ps_small", bufs=2, space="PSUM") as ps_small,
        ):
          for _rep in range(repeats):
            ident = sb.tile([128, 128], BF16, tag="ident", bufs=1)
            make_identity(nc, ident)

            wq1, _ = _load_weight2d(nc, sb, scratch, "Wq1", w_d["Wq1"], F, H * U1, False)
            wk1, _ = _load_weight2d(nc, sb, scratch, "Wk1", w_d["Wk1"], F, H * U1, False)
            # ---- XT [F, N] bf16 via PE transpose ----
            x32 = sb.tile([128, NT, F], F32, tag="x32", bufs=1)
            nc.sync.dma_start(out=x32, in_=x_d[:].rearrange("(t p) f -> p t f", p=128))
            xb = sb.tile([128, NT, F], BF16, tag="xb", bufs=1)
            nc.vector.tensor_copy(out=xb, in_=x32)
            XT = sb.tile([F, N], BF16, tag="XT", bufs=1)
            for t in range(NT):
                pt = ps_small.tile([F, 128], BF16, tag="ps_c", bufs=2)
                nc.tensor.transpose(pt, xb[:, t, :], ident)
                nc.scalar.copy(out=XT[:, t * 128:(t + 1) * 128], in_=pt)

            # ---- A mask prep (DMA-heavy; scheduled before bulk weights) ----
            mn = sb.tile([128, NT, N], BF16, tag="E", bufs=4)
            for cc in range(NT):
                a32c = scratch.tile([128, N], F32, tag="a32c", bufs=2)
                nc.sync.dma_start(
                    out=a32c,
                    in_=a_d[:].rearrange("(t p) n -> p t n", p=128)[:, cc, :])
                nc.vector.tensor_scalar(out=mn[:, cc, :], in0=a32c, scalar1=-1.0,
                                        scalar2=1.0, op0=OP.mult, op1=OP.add)
            wv1h, wv1l = _load_weight2d(nc, sb, scratch, "Wv1", w_d["Wv1"], F, H * U1, True)
            wo1h, wo1l = _load_weight_kt(nc, sb, scratch, "Wo1", w_d["Wo1"], H * U1 // 128, U1, True)
            Mt = sb.tile([128, NT, N], BF16, tag="Mt", bufs=1)
            for cc in range(NT):
                nc.sync.dma_start(out=Mt[:, :, cc * 128:(cc + 1) * 128],
                                  in_=mn[:, cc, :], transpose=True)

            # ---- layer-2 weights loaded lazily inside the layer-1 loop ----
            dw = {}

            def deferred_loads(h):
                if h == 2:
                    dw['q2'] = _load_weight2d(nc, sb, scratch, "Wq2",
                                              w_d["Wq2"], U1, H * U2, False)[0]
                    dw['k2'] = _load_weight2d(nc, sb, scratch, "Wk2",
                                              w_d["Wk2"], U1, H * U2, False)[0]
                elif h == 3:
                    dw['v2'] = _load_weight2d(nc, sb, scratch, "Wv2",
                                              w_d["Wv2"], U1, H * U2, True)
                elif h == 4:
                    dw['o2'] = _load_weight_kt(nc, sb, scratch, "Wo2",
                                               w_d["Wo2"], H * U2 // 128, U2,
                                               True)
                elif h == 5:
                    for nm, shp in (("W1", [F, 32]), ("W2", [32, 16]),
                                    ("W3", [16, 1])):
                        t = sb.tile(shp, F32, tag=nm, bufs=1, name=nm)
                        nc.sync.dma_start(out=t, in_=w_d[nm][:])
                        dw[nm] = t
                    for nm, pp in (("b1", 32), ("b2", 16), ("b3", 1)):
                        t = sb.tile([pp, 1], F32, tag=nm, bufs=1, name=nm)
                        nc.sync.dma_start(
                            out=t,
                            in_=w_d[nm][:].rearrange("(p x) -> p x", x=1))
                        dw[nm] = t

            # ---- layers ----
            H1T = sb.tile([U1, N], BF16, tag="H1T", bufs=1)
            _gat_layer(nc, sb, ps_big, ps_small, XT, U1, U1,
                       wq1, wk1, wv1h, wv1l, wo1h, wo1l, Mt, H1T,
                       deferred_cb=deferred_loads)
            wq2, wk2 = dw['q2'], dw['k2']
            wv2h, wv2l = dw['v2']
            wo2h, wo2l = dw['o2']
            w1, w2, w3 = dw['W1'], dw['W2'], dw['W3']
            b1, b2, b3 = dw['b1'], dw['b2'], dw['b3']
            H2T = sb.tile([U2, N], BF16, tag="H2T", bufs=1)
            _gat_layer(nc, sb, ps_big, ps_small, H1T, U2, U2,
                       wq2, wk2, wv2h, wv2l, wo2h, wo2l, Mt, H2T)

            # ---- mean pool + MLP ----
            hs2 = sb.tile([U2, 2], F32, tag="hsum2", bufs=1)
            for q in range(2):
                nc.vector.reduce_sum(out=hs2[:, q:q + 1],
                                     in_=H2T[:, q * 512:(q + 1) * 512],
                                     axis=mybir.AxisListType.X)
            hs = sb.tile([U2, 1], F32, tag="hsum", bufs=1)
            nc.vector.tensor_add(out=hs, in0=hs2[:, 0:1], in1=hs2[:, 1:2])
            p1 = ps_small.tile([32, 1], F32, tag="ps_c", bufs=2)
            nc.tensor.matmul(p1, lhsT=w1, rhs=hs, start=True, stop=True)
            a1 = sb.tile([32, 1], F32, tag="a1", bufs=1)
            nc.scalar.activation(out=a1, in_=p1, func=AF.Relu, bias=b1,
                                 scale=1.0 / float(N))
            p2 = ps_small.tile([16, 1], F32, tag="ps_c", bufs=2)
            nc.tensor.matmul(p2, lhsT=w2, rhs=a1, start=True, stop=True)
            a2 = sb.tile([16, 1], F32, tag="a2", bufs=1)
            nc.scalar.activation(out=a2, in_=p2, func=AF.Relu, bias=b2)
            p3 = ps_small.tile([1, 1], F32, tag="ps_c", bufs=2)
            nc.tensor.matmul(p3, lhsT=w3, rhs=a2, start=True, stop=True)
            yt = sb.tile([1, 1], F32, tag="yt", bufs=1)
            nc.vector.tensor_add(out=yt, in0=p3, in1=b3)
            nc.sync.dma_start(out=y_d[:], in_=yt)

    nc.compile()
    return nc


_NC = None


def _get_nc():
    global _NC
    if _NC is None:
        _NC = build_nc()
    return _NC


def make_in_maps(inputs):
    in_maps = []
    for i in range(B):
        m = {"X": np.ascontiguousarray(np.asarray(inputs["X"][i], dtype=np.float32)),
             "A": np.ascontiguousarray(np.asarray(inputs["A"][i], dtype=np.float32))}
        for k in WEIGHT_NAMES:
            m[k] = np.ascontiguousarray(np.asarray(inputs[k], dtype=np.float32))
        in_maps.append(m)
    return in_maps


def run(inputs, trace=False):
    nc = _get_nc()
    res = run_bass_kernel_spmd(nc, make_in_maps(inputs), list(range(B)), trace=trace)
    y = np.stack([res.results[i]["y"][0] for i in range(B)], axis=0)
    return y.astype(np.float32), res


def kernel(**inputs):
    y, _ = run(inputs, trace=False)
    return y



# revision 26
# speedup vs baseline: 1.1783x; 1.1783x over previous
"""Trainium2 Bass kernel for nn_CustomGNN_66881230733874 (2-layer GAT + mean-pool + MLP).

Sharding: data-parallel over batch B=8 -> one graph per NeuronCore (8 cores).
Each core computes its full graph end-to-end (no collectives); host gathers [8,1].

Layout strategy (per core):
  - Activations live feature-on-partition ("transposed"): XT [F, N], QT/KT [(h,d), N].
  - Scores computed directly transposed: S^T[m, n] = K_h Q_h^T via PE.
  - softmax without max-subtraction (scores are O(1) by construction);
    masked entries are exactly zeroed by multiplying exp(S) with (1-A)^T.
  - ctx_unnorm [n, u] via lhsT=E[m,n-chunk], rhs=V_aug[m, u+1] where V_aug has a
    ones column -> last PSUM column accumulates the softmax denominator Z[n].
  - normalize with per-partition reciprocal (DVE), pack bf16, DMA-transpose
    (SBUF xbar) to get ctx^T [(h,u), N] for the W_out matmul. No PE transposes.
  - Weight matmuls use split-bf16 (hi+lo) for the value path (Wv, Wo) to kill
    correlated rounding error; Wq/Wk single-bf16 (score path is tolerant).
"""

import numpy as np

import concourse.bass as bass
import concourse.mybir as mybir
import concourse.tile as tile
from concourse import bacc
from concourse.bass_utils import run_bass_kernel_spmd
from concourse.masks import make_identity

F32 = mybir.dt.float32
BF16 = mybir.dt.bfloat16
AF = mybir.ActivationFunctionType
OP = mybir.AluOpType

B = 8
N = 1024
F = 64
H = 8
U1, U2 = 128, 64
NT = N // 128  # 8 node chunks

WEIGHT_NAMES = [
    "Wq1", "Wk1", "Wv1", "Wo1", "Wq2", "Wk2", "Wv2", "Wo2",
    "W1", "b1", "W2", "b2", "W3", "b3",
]


def _load_weight2d(nc, sb, scratch, name, dram, part, cols, split):
    """DMA a [part, cols] f32 weight to SBUF bf16 (hi[, lo])."""
    w32 = scratch.tile([part, cols], F32, tag="wscratch", bufs=2)
    nc.sync.dma_start(out=w32, in_=dram[:])
    hi = sb.tile([part, cols], BF16, tag=f"w_{name}_hi", bufs=1)
    nc.vector.tensor_copy(out=hi, in_=w32)
    lo = None
    if split:
        lo = sb.tile([part, cols], BF16, tag=f"w_{name}_lo", bufs=1)
        nc.vector.tensor_sub(out=lo, in0=w32, in1=hi)
    return hi, lo


def _load_weight_kt(nc, sb, scratch, name, dram, kt, cols, split):
    """DMA a [kt*128, cols] f32 weight to SBUF bf16 [128, kt, cols] (hi, lo)."""
    w32 = scratch.tile([128, kt, cols], F32, tag="wscratch", bufs=2)
    nc.sync.dma_start(out=w32, in_=dram[:].rearrange("(k p) c -> p k c", p=128))
    hi = sb.tile([128, kt, cols], BF16, tag=f"w_{name}_hi", bufs=1)
    nc.vector.tensor_copy(out=hi, in_=w32)
    lo = sb.tile([128, kt, cols], BF16, tag=f"w_{name}_lo", bufs=1)
    nc.vector.tensor_sub(out=lo, in0=w32, in1=hi)
    return hi, lo


def _gat_layer(nc, sb, ps_big, ps_small, XT, U, Uout,
               Wq, Wk, Wv_hi, Wv_lo, Wo_hi, Wo_lo, Mt, HT_out):
    """One GAT layer. XT: bf16 [Cin, N]. Writes HT_out: bf16 [Uout, N]."""
    HU = H * U
    DT = HU // 128          # number of 128-row tiles of QT/KT
    UD = 128 // U           # heads per QT/KT tile
    VW = U + 1              # V_aug width per head
    inv_sqrt_u = 1.0 / float(np.sqrt(U))

    # ---- projections: QT/KT [(h,d), N] bf16; V_aug [n, H*(U+1)] bf16 ----
    QT = sb.tile([128, DT, N], BF16, tag="QT", bufs=1)
    KT = sb.tile([128, DT, N], BF16, tag="KT", bufs=1)
    VA = sb.tile([128, NT, H * VW], BF16, tag="VA", bufs=1)

    def qk_proj(d, tag="ps_p", pool=None):
        for w, dst, on_act in ((Wq, QT, True), (Wk, KT, False)):
            for q in range(2):
                p = (pool or ps_big).tile([128, 512], F32, tag=tag, bufs=2)
                nc.tensor.matmul(p, lhsT=w[:, d * 128:(d + 1) * 128],
                                 rhs=XT[:, q * 512:(q + 1) * 512],
                                 start=True, stop=True)
                if on_act:
                    nc.scalar.copy(out=dst[:, d, q * 512:(q + 1) * 512], in_=p)
                else:
                    nc.vector.tensor_copy(out=dst[:, d, q * 512:(q + 1) * 512],
                                          in_=p)

    def v_proj():
        n_vc = HU // 512  # 512-wide chunks of H*U
        hpc = 512 // U    # heads per 512 chunk
        for m in range(NT):
            VAm = VA[:, m, :].rearrange("p (h x) -> p h x", h=H)
            nc.vector.memset(VAm[:, :, U:U + 1], 1.0)
        for c in range(n_vc):
            for m in range(NT):
                VAm = VA[:, m, :].rearrange("p (h x) -> p h x", h=H)
                p = ps_big.tile([128, 512], F32, tag="ps_p", bufs=2)
                xm = XT[:, m * 128:(m + 1) * 128]
                nc.tensor.matmul(p, lhsT=xm, rhs=Wv_hi[:, c * 512:(c + 1) * 512],
                                 start=True, stop=False)
                nc.tensor.matmul(p, lhsT=xm, rhs=Wv_lo[:, c * 512:(c + 1) * 512],
                                 start=False, stop=True)
                nc.vector.tensor_copy(
                    out=VAm[:, c * hpc:(c + 1) * hpc, 0:U],
                    in_=p.rearrange("p (h x) -> p h x", h=hpc),
                )

    # ---- attention, software-pipelined: S(h+1) emitted before ctx(h) so the
    # exp stream on ACT is never starved by PE's ctx block. m-chunks are
    # processed in pairs (FD=2048) to halve per-instruction overheads ----
    CT = sb.tile([128, DT, N], BF16, tag="CT", bufs=1)  # ctx^T [(h,u), N]
    E_tiles = {}
    cp2_tiles = {}
    wo_psum = [None, None]
    KD = HU // 128

    def s_phase(h):
        kt_i, k_off = h // UD, (h % UD) * U
        E = sb.tile([128, NT, N], BF16, tag="E", bufs=4)
        E_tiles[h] = E
        for m in range(NT):
            s = ps_big.tile([128, 1024], F32, tag="ps_s", bufs=2)
            for q in range(2):
                nc.tensor.matmul(
                    s[:, q * 512:(q + 1) * 512],
                    lhsT=KT[k_off:k_off + U, kt_i, m * 128:(m + 1) * 128],
                    rhs=QT[k_off:k_off + U, kt_i, q * 512:(q + 1) * 512],
                    start=True, stop=True,
                )
            nc.scalar.activation(out=E[:, m, :], in_=s, func=AF.Exp,
                                 scale=inv_sqrt_u)
            nc.vector.tensor_mul(out=E[:, m, :], in0=E[:, m, :], in1=Mt[:, m, :])

    GH = 2                    # heads per transpose/Wo group
    TPG = GH * U // 128       # CT tiles per group

    def ctx_phase(h):
        E = E_tiles.pop(h)
        g, hg = divmod(h, GH)
        batched_t = (U == 128)  # head-major cpq -> one xbar transpose per group
        if hg == 0:
            if batched_t:
                cpq_new = sb.tile([128, GH, NT, U], BF16, tag="cpq", bufs=2)
            else:
                cpq_new = sb.tile([128, NT, GH * U], BF16, tag="cpq", bufs=2)
            cp2_tiles[g] = cpq_new
        cpq = cp2_tiles[g]
        for nn in range(NT):
            c = ps_small.tile([128, VW], F32, tag="ps_c", bufs=2)
            for m in range(NT):
                nc.tensor.matmul(
                    c,
                    lhsT=E[:, m, nn * 128:(nn + 1) * 128],
                    rhs=VA[:, m, h * VW:(h + 1) * VW],
                    start=(m == 0), stop=(m == NT - 1),
                )
            r = sb.tile([128, 1], F32, tag="recip", bufs=8)
            nc.vector.reciprocal(out=r, in_=c[:, U:U + 1])
            dst = (cpq[:, hg, nn, :] if batched_t
                   else cpq[:, nn, hg * U:(hg + 1) * U])
            nc.vector.tensor_scalar(out=dst, in0=c[:, 0:U], scalar1=r,
                                    scalar2=None, op0=OP.mult)
        # one xbar transpose per group: free index c lands at
        # out[c%128, c//128, p]; both layouts put (head-row, tile) in
        # (c%128 resp. c//128) so the same CT view works.
        if hg == GH - 1:
            nc.sync.dma_start(
                out=CT[:, g * TPG:(g + 1) * TPG, :].rearrange(
                    "p k (t c) -> p k t c", c=128),
                in_=(cpq[:, :, :, :] if batched_t else cpq[:, :, :]),
                transpose=True)
        # fold the Wo accumulation once this group's CT tiles are complete
        if hg == GH - 1:
            first, last = (g == 0), (g == H // GH - 1)
            for q in range(2):
                if first:
                    wp = ps_big.tile([Uout, 512], F32, tag="ps_p", bufs=2)
                    wo_psum[q] = wp
                for ki in range(TPG):
                    k = g * TPG + ki
                    rhs = CT[:, k, q * 512:(q + 1) * 512]
                    nc.tensor.matmul(wo_psum[q], lhsT=Wo_hi[:, k, :], rhs=rhs,
                                     start=(first and ki == 0), stop=False)
                    nc.tensor.matmul(wo_psum[q], lhsT=Wo_lo[:, k, :], rhs=rhs,
                                     start=False, stop=(last and ki == TPG - 1))
            if last:
                for q in range(2):
                    nc.scalar.copy(out=HT_out[:, q * 512:(q + 1) * 512],
                                   in_=wo_psum[q])

    # head-0's QK projection chunk first so its S/exp stream starts early;
    # the remaining projections overlap the first heads' attention
    emitted_qk = set()

    def qk_if_needed(h, **kw):
        d = h // UD
        if h < H and d not in emitted_qk:
            emitted_qk.add(d)
            qk_proj(d, **kw)

    qk_if_needed(0)
    qk_if_needed(1)
    v_proj()
    qk_if_needed(2, tag="ps_c", pool=ps_small)
    s_phase(0)
    s_phase(1)
    for h in range(H):
        if h + 2 < H:
            qk_if_needed(h + 3, tag="ps_c", pool=ps_small)
            s_phase(h + 2)
        ctx_phase(h)


def build_nc(repeats=1):
    nc = bacc.Bacc("TRN2", target_bir_lowering=False, debug=False)

    x_d = nc.dram_tensor("X", [N, F], F32, kind="ExternalInput")
    a_d = nc.dram_tensor("A", [N, N], F32, kind="ExternalInput")
    w_d = {}
    shapes = {
        "Wq1": [F, H * U1], "Wk1": [F, H * U1], "Wv1": [F, H * U1],
        "Wo1": [H * U1, U1],
        "Wq2": [U1, H * U2], "Wk2": [U1, H * U2], "Wv2": [U1, H * U2],
        "Wo2": [H * U2, U2],
        "W1": [F, 32], "b1": [32], "W2": [32, 16], "b2": [16],
        "W3": [16, 1], "b3": [1],
    }
    for k, s in shapes.items():
        w_d[k] = nc.dram_tensor(k, s, F32, kind="ExternalInput")
    y_d = nc.dram_tensor("y", [1, 1], F32, kind="ExternalOutput")

    with tile.TileContext(nc) as tc:
        with (
            tc.tile_pool(name="sb", bufs=1) as sb,
            tc.tile_pool(name="scratch", bufs=2) as scratch,
            tc.tile_pool(name="ps_big", bufs=2, space="PSUM") as ps_big,
            tc.tile_pool(name="ps_small", bufs=2, space="PSUM") as ps_small,
        ):
          for _rep in range(repeats):
            ident = sb.tile([128, 128], BF16, tag="ident", bufs=1)
            make_identity(nc, ident)

            wq1, _ = _load_weight2d(nc, sb, scratch, "Wq1", w_d["Wq1"], F, H * U1, False)
            wk1, _ = _load_weight2d(nc, sb, scratch, "Wk1", w_d["Wk1"], F, H * U1, False)
            # ---- XT [F, N] bf16 via PE transpose ----
            x32 = sb.tile([128, NT, F], F32, tag="x32", bufs=1)
            nc.sync.dma_start(out=x32, in_=x_d[:].rearrange("(t p) f -> p t f", p=128))
            xb = sb.tile([128, NT, F], BF16, tag="xb", bufs=1)
            nc.vector.tensor_copy(out=xb, in_=x32)
            XT = sb.tile([F, N], BF16, tag="XT", bufs=1)
            for t in range(NT):
                pt = ps_small.tile([F, 128], BF16, tag="ps_c", bufs=2)
                nc.tensor.transpose(pt, xb[:, t, :], ident)
                nc.scalar.copy(out=XT[:, t * 128:(t + 1) * 128], in_=pt)

            # ---- A mask prep (DMA-heavy; scheduled before bulk weights) ----
            mn = sb.tile([128, NT, N], BF16, tag="E", bufs=4)
            for cc in range(NT):
                a32c = scratch.tile([128, N], F32, tag="a32c", bufs=2)
                nc.sync.dma_start(
                    out=a32c,
                    in_=a_d[:].rearrange("(t p) n -> p t n", p=128)[:, cc, :])
                nc.vector.tensor_scalar(out=mn[:, cc, :], in0=a32c, scalar1=-1.0,
                                        scalar2=1.0, op0=OP.mult, op1=OP.add)
            wv1h, wv1l = _load_weight2d(nc, sb, scratch, "Wv1", w_d["Wv1"], F, H * U1, True)
            wo1h, wo1l = _load_weight_kt(nc, sb, scratch, "Wo1", w_d["Wo1"], H * U1 // 128, U1, True)
            Mt = sb.tile([128, NT, N], BF16, tag="Mt", bufs=1)
            for cc in range(NT):
                nc.sync.dma_start(out=Mt[:, :, cc * 128:(cc + 1) * 128],
                                  in_=mn[:, cc, :], transpose=True)

            # ---- weights ----
            wq2, _ = _load_weight2d(nc, sb, scratch, "Wq2", w_d["Wq2"], U1, H * U2, False)
            wk2, _ = _load_weight2d(nc, sb, scratch, "Wk2", w_d["Wk2"], U1, H * U2, False)
            wv2h, wv2l = _load_weight2d(nc, sb, scratch, "Wv2", w_d["Wv2"], U1, H * U2, True)
            wo2h, wo2l = _load_weight_kt(nc, sb, scratch, "Wo2", w_d["Wo2"], H * U2 // 128, U2, True)

            w1 = sb.tile([F, 32], F32, tag="W1", bufs=1)
            nc.sync.dma_start(out=w1, in_=w_d["W1"][:])
            w2 = sb.tile([32, 16], F32, tag="W2", bufs=1)
            nc.sync.dma_start(out=w2, in_=w_d["W2"][:])
            w3 = sb.tile([16, 1], F32, tag="W3", bufs=1)
            nc.sync.dma_start(out=w3, in_=w_d["W3"][:])
            b1 = sb.tile([32, 1], F32, tag="b1", bufs=1)
            nc.sync.dma_start(out=b1, in_=w_d["b1"][:].rearrange("(p x) -> p x", x=1))
            b2 = sb.tile([16, 1], F32, tag="b2", bufs=1)
            nc.sync.dma_start(out=b2, in_=w_d["b2"][:].rearrange("(p x) -> p x", x=1))
            b3 = sb.tile([1, 1], F32, tag="b3", bufs=1)
            nc.sync.dma_start(out=b3, in_=w_d["b3"][:].rearrange("(p x) -> p x", x=1))

            # ---- layers ----
            H1T = sb.tile([U1, N], BF16, tag="H1T", bufs=1)
            _gat_layer(nc, sb, ps_big, ps_small, XT, U1, U1,
                       wq1, wk1, wv1h, wv1l, wo1h, wo1l, Mt, H1T)
            H2T = sb.tile([U2, N], BF16, tag="H2T", bufs=1)
            _gat_layer(nc, sb, ps_big, ps_small, H1T, U2, U2,
                       wq2, wk2, wv2h, wv2l, wo2h, wo2l, Mt, H2T)

            # ---- mean pool + MLP ----
            hs2 = sb.tile([U2, 2], F32, tag="hsum2", bufs=1)
            for q in range(2):
                nc.vector.reduce_sum(out=hs2[:, q:q + 1],
                                     in_=H2T[:, q * 512:(q + 1) * 512],
                                     axis=mybir.AxisListType.X)
            hs = sb.tile([U2, 1], F32, tag="hsum", bufs=1)
            nc.vector.tensor_add(out=hs, in0=hs2[:, 0:1], in1=hs2[:, 1:2])
            p1 = ps_small.tile([32, 1], F32, tag="ps_c", bufs=2)
            nc.tensor.matmul(p1, lhsT=w1, rhs=hs, start=True, stop=True)
            a1 = sb.tile([32, 1], F32, tag="a1", bufs=1)
            nc.scalar.activation(out=a1, in_=p1, func=AF.Relu, bias=b1,
                                 scale=1.0 / float(N))
            p2 = ps_small.tile([16, 1], F32, tag="ps_c", bufs=2)
            nc.tensor.matmul(p2, lhsT=w2, rhs=a1, start=True, stop=True)
            a2 = sb.tile([16, 1], F32, tag="a2", bufs=1)
            nc.scalar.activation(out=a2, in_=p2, func=AF.Relu, bias=b2)
            p3 = ps_small.tile([1, 1], F32, tag="ps_c", bufs=2)
            nc.tensor.matmul(p3, lhsT=w3, rhs=a2, start=True, stop=True)
            yt = sb.tile([1, 1], F32, tag="yt", bufs=1)
            nc.vector.tensor_add(out=yt, in0=p3, in1=b3)
            nc.sync.dma_start(out=y_d[:], in_=yt)

    nc.compile()
    return nc


_NC = None


def _get_nc():
    global _NC
    if _NC is None:
        _NC = build_nc()
    return _NC


def make_in_maps(inputs):
    in_maps = []
    for i in range(B):
        m = {"X": np.ascontiguousarray(np.asarray(inputs["X"][i], dtype=np.float32)),
             "A": np.ascontiguousarray(np.asarray(inputs["A"][i], dtype=np.float32))}
        for k in WEIGHT_NAMES:
            m[k] = np.ascontiguousarray(np.asarray(inputs[k], dtype=np.float32))
        in_maps.append(m)
    return in_maps


def run(inputs, trace=False):
    nc = _get_nc()
    res = run_bass_kernel_spmd(nc, make_in_maps(inputs), list(range(B)), trace=trace)
    y = np.stack([res.results[i]["y"][0] for i in range(B)], axis=0)
    return y.astype(np.float32), res


def kernel(**inputs):
    y, _ = run(inputs, trace=False)
    return y

